# revision 2
# baseline (speedup 1.0000x reference)
"""Trainium2 Bass kernel for causal self-attention (B=2, S=2048, D=1024, H=16).

Sharding: 8 cores = 2 batches x 4 head-groups (4 heads / 256 channels each).
Each core computes the qkv projection for its head block, causal attention for
its 4 heads, and a partial output projection (contraction over its 256
channels). The host sums the 4 partials per batch and adds b_out at gather.

Schedule: the kernel is ACT(exp)-bound in the attention phase (~74us of exp)
while QKV projection is pure PE work (~41us). QKV is therefore interleaved
per q-chunk in ASCENDING order so the exp chain starts after ~1/4 of the
projection instead of all of it, and outproj/next-chunk QKV matmuls fill PE
gaps under the exp chain.

Device dataflow (bf16 matmuls, fp32 accumulation):
  - x is fed pre-transposed (xT [D,S]) so every matmul has its contraction on
    partitions with no on-device transposes.
  - Q^T/K^T computed channel-major [ch, t]; V token-major [t, j] with a ones
    column appended -> the attention matmul produces numerator rows 0..63 and
    the softmax denominator in PSUM row 64 in one accumulation group.
  - scores computed transposed (k on partitions, q on free) so exp/mask/AV
    all chain without transposes; softmax needs no max-subtraction (scores
    are O(1) by construction) and division is deferred past the AV matmul.
  - causal: only k-tiles at/below the diagonal are computed; the 4 diagonal
    tiles per q-chunk restrict to their valid column range and apply a
    precomputed multiplicative 0/1 mask (on the Pool engine - DVE is loaded).
  - head pairs are packed into the 128-row PE array (base partitions 0/64,
    row-tiled 64x128 so the two heads' score matmuls run concurrently).
  - k-tiles are processed in PAIRS: scores for two k-tiles land in one
    4-bank PSUM quad and off-diagonal pairs get ONE exp instruction
    (halves the ~352-cycle ACT per-instruction overhead).
"""

import sys

if "/opt/trn_rl_repo" not in sys.path:
    sys.path.insert(0, "/opt/trn_rl_repo")

import numpy as np
import ml_dtypes

import concourse.mybir as mybir
import concourse.tile as tile
from concourse import bacc

B, S, D, H, DK = 2, 2048, 1024, 16, 64
N_CORES = 8
HPC = 4  # heads per core
DH = HPC * DK  # 256 channels per core
P = 128
QC = 512  # q-chunk width
NQC = S // QC  # 4
NKT = S // P  # 16 k-tiles
DT = D // P  # 8 d-tiles
SCALE = 1.0 / np.sqrt(DK)

BF16 = mybir.dt.bfloat16
F32 = mybir.dt.float32


def build_nc(n_cores: int = N_CORES, repeats: int = 1):
    nc = bacc.Bacc("TRN2", target_bir_lowering=False, debug=False, num_devices=n_cores)

    xT = nc.dram_tensor("xT", [D, S], BF16, kind="ExternalInput")
    wq = nc.dram_tensor("wq", [D, DH], BF16, kind="ExternalInput")
    wk = nc.dram_tensor("wk", [D, DH], BF16, kind="ExternalInput")
    wv = nc.dram_tensor("wv", [D, DH], BF16, kind="ExternalInput")
    wo = nc.dram_tensor("wo", [DH, D], BF16, kind="ExternalInput")
    bq = nc.dram_tensor("bq", [2, P], F32, kind="ExternalInput")
    bk = nc.dram_tensor("bk", [2, P], F32, kind="ExternalInput")
    bv = nc.dram_tensor("bv", [1, DH], F32, kind="ExternalInput")
    y = nc.dram_tensor("y", [S, D], F32, kind="ExternalOutput")

    with tile.TileContext(nc) as tc:
        for _ in range(repeats):
            _body(nc, tc, xT, wq, wk, wv, wo, bq, bk, bv, y)

    nc.compile()
    return nc


def _body(nc, tc, xT, wq, wk, wv, wo, bq, bk, bv, y):
    add = mybir.AluOpType.add
    mult = mybir.AluOpType.mult
    Exp = mybir.ActivationFunctionType.Exp

    xT_r = xT.ap().rearrange("(dt p) t -> p dt t", p=P)
    wq_r = wq.ap().rearrange("(dt p) c -> p dt c", p=P)
    wk_r = wk.ap().rearrange("(dt p) c -> p dt c", p=P)
    wv_r = wv.ap().rearrange("(dt p) c -> p dt c", p=P)

    with (
        tc.tile_pool(name="const", bufs=1) as const,
        tc.tile_pool(name="work", bufs=6) as work,
        tc.tile_pool(name="psum", bufs=1, space="PSUM") as psum,
    ):
        # ---- persistent SBUF state (DMA split per chunk-slice so compute
        # can start as soon as the first slices land) ----
        xT_sb = const.tile([P, DT, S], BF16)
        wq_sb = const.tile([P, DT, DH], BF16)
        wk_sb = const.tile([P, DT, DH], BF16)
        wv_sb = const.tile([P, DT, DH], BF16)
        for dt in range(DT):
            nc.sync.dma_start(wk_sb[:, dt], wk_r[:, dt])
            nc.sync.dma_start(wq_sb[:, dt], wq_r[:, dt])
            nc.sync.dma_start(wv_sb[:, dt], wv_r[:, dt])
            nc.sync.dma_start(xT_sb[:, dt, 0:QC], xT_r[:, dt, 0:QC])
        for c in range(1, NQC):
            for dt in range(DT):
                nc.sync.dma_start(
                    xT_sb[:, dt, c * QC : (c + 1) * QC],
                    xT_r[:, dt, c * QC : (c + 1) * QC],
                )

        wo_sb = const.tile([P, 2, D], BF16)
        nc.sync.dma_start(wo_sb[:], wo.ap().rearrange("(ht p) e -> p ht e", p=P))

        bq_sb = const.tile([P, 2], F32)
        bk_sb = const.tile([P, 2], F32)
        nc.sync.dma_start(bq_sb[:], bq.ap().rearrange("mt p -> p mt"))
        nc.sync.dma_start(bk_sb[:], bk.ap().rearrange("mt p -> p mt"))

        bv_bc = const.tile([P, DH], F32)
        nc.sync.dma_start(bv_bc[0:1, :], bv.ap())
        nc.gpsimd.partition_broadcast(bv_bc[:], bv_bc[0:1, :])

        # causal masks for the 4 diagonal k-tiles of each q-chunk:
        # mask[p, i, ql] = 1.0 if p <= ql - 128*i else 0.0
        mask_sb = const.tile([P, 4, QC], BF16)
        nc.vector.memset(mask_sb[:], 1.0)
        for i in range(4):
            nc.gpsimd.affine_select(
                out=mask_sb[:, i, :],
                in_=mask_sb[:, i, :],
                compare_op=mybir.AluOpType.is_ge,
                fill=0.0,
                base=-P * i,
                pattern=[[1, QC]],
                channel_multiplier=-1,
            )

        qT_sb = const.tile([P, 2, S], BF16)  # [ch within mtile, mtile, t]
        kT_sb = const.tile([P, 2, S], BF16)
        # V' [t-part, ktile, head, dk+1]; col DK holds ones (softmax denom)
        vp_sb = const.tile([P, NKT, HPC, DK + 1], BF16)
        nc.vector.memset(vp_sb[:, :, :, DK : DK + 1], 1.0)
        aT_sb = const.tile([P, 2, S], BF16)  # attention out, channel-major

        def proj_chunk(wsb, bsb, dst, c):
            # K^T / Q^T channel-major for q-columns of chunk c:
            # psum[ch, t] += w[d, ch].T @ xT[d, t]
            cs = slice(c * QC, (c + 1) * QC)
            for mt in range(2):
                ps = psum.tile([P, QC], F32, tag="pp", bufs=2)
                for dt in range(DT):
                    nc.tensor.matmul(
                        ps[:],
                        lhsT=wsb[:, dt, mt * P : (mt + 1) * P],
                        rhs=xT_sb[:, dt, cs],
                        start=(dt == 0),
                        stop=(dt == DT - 1),
                    )
                nc.vector.tensor_scalar(
                    dst[:, mt, cs], ps[:], bsb[:, mt : mt + 1], None, op0=add
                )

        def vproj_tile(kt):
            # V token-major: psum[t, j] += xT[d, t-tile].T @ wv[d, j]
            ps = psum.tile([P, QC], F32, tag="pp", bufs=2)
            for dt in range(DT):
                nc.tensor.matmul(
                    ps[:, 0:DH],
                    lhsT=xT_sb[:, dt, kt * P : (kt + 1) * P],
                    rhs=wv_sb[:, dt, :],
                    start=(dt == 0),
                    stop=(dt == DT - 1),
                )
            nc.vector.tensor_tensor(
                vp_sb[:, kt, :, 0:DK],
                ps[:, 0:DH].rearrange("p (h j) -> p h j", j=DK),
                bv_bc[:].rearrange("p (h j) -> p h j", j=DK),
                add,
            )

        # ---- attention for one head-pair and q-chunk ----
        # k-tiles processed in pairs sharing a 4-bank PSUM quad; off-diag
        # pairs get a single batched exp instruction.
        def attention(hp, qc):
            nkt = 4 * (qc + 1)  # causal: k-tiles 0 .. 4*qc+3
            qs = slice(qc * QC, (qc + 1) * QC)
            av = psum.tile([DK + 1, 2, QC], F32, tag="av", name="av")
            for pi in range(nkt // 2):
                kts = (2 * pi, 2 * pi + 1)
                scq = psum.tile([P, 2, 2, QC], F32, tag="scq", name="scq")
                for i, kt in enumerate(kts):
                    diag = kt - 4 * qc
                    cl = max(0, diag) * P
                    for hh in range(2):
                        lo, hi = hh * DK, (hh + 1) * DK
                        nc.tensor.matmul(
                            scq[:, i, hh, cl:QC],
                            lhsT=kT_sb[lo:hi, hp, kt * P : (kt + 1) * P],
                            rhs=qT_sb[lo:hi, hp, qc * QC + cl : (qc + 1) * QC],
                            start=True,
                            stop=True,
                        )
                ex = work.tile([P, 2, 2, QC], BF16, tag="exp", bufs=4)
                if kts[1] < 4 * qc:
                    # both off-diagonal: one exp over the whole quad
                    nc.scalar.activation(ex[:], scq[:], Exp, scale=SCALE)
                else:
                    for i, kt in enumerate(kts):
                        diag = kt - 4 * qc
                        cl = max(0, diag) * P
                        nc.scalar.activation(
                            ex[:, i, :, cl:QC], scq[:, i, :, cl:QC], Exp, scale=SCALE
                        )
                        if diag >= 0:
                            # causal mask on the Pool engine (SBUF-only op)
                            nc.gpsimd.tensor_tensor(
                                ex[:, i, :, cl:QC],
                                ex[:, i, :, cl:QC],
                                mask_sb[:, diag : diag + 1, cl:QC].to_broadcast(
                                    (P, 2, QC - cl)
                                ),
                                mult,
                            )
                for i, kt in enumerate(kts):
                    diag = kt - 4 * qc
                    cl = max(0, diag) * P
                    for hh in range(2):
                        nc.tensor.matmul(
                            av[:, hh, cl:QC],
                            lhsT=vp_sb[:, kt, 2 * hp + hh, :],
                            rhs=ex[:, i, hh, cl:QC],
                            start=(kt == 0),
                            stop=(kt == nkt - 1),
                        )

            # softmax division: row DK of av is the denominator; reciprocal
            # on partition DK, DMA the row to partition 0 (gpsimd broadcast
            # only honours base partition 0 on HW), broadcast down, multiply.
            rec = work.tile([P, 2, QC], F32, tag="rec", bufs=2)
            nc.vector.reciprocal(rec[DK : DK + 1, :, :], av[DK : DK + 1, :, :])
            bcs = work.tile([1, 2, QC], F32, tag="bcs", bufs=2)
            nc.sync.dma_start(bcs[0:1, :, :], rec[DK : DK + 1, :, :])
            bc = work.tile([DK, 2, QC], F32, tag="bc")
            nc.gpsimd.partition_broadcast(bc[:], bcs[0:1, :, :])
            st = work.tile([DK, 2, QC], BF16, tag="st")
            nc.vector.tensor_mul(st[:], av[0:DK, :, :], bc[:])
            nc.sync.dma_start(aT_sb[0:DK, hp, qs], st[:, 0, :])
            nc.sync.dma_start(aT_sb[DK:P, hp, qs], st[:, 1, :])

        def outproj(qc):
            # partial y for t-tiles of chunk qc; b_out is added on the host
            for tt in range(4 * qc, 4 * qc + 4):
                ysb = work.tile([P, D], F32, tag="y")
                for ec in range(2):
                    ps = psum.tile([P, QC], F32, tag="pp", bufs=2)
                    for ht in range(2):
                        nc.tensor.matmul(
                            ps[:],
                            lhsT=aT_sb[:, ht, tt * P : (tt + 1) * P],
                            rhs=wo_sb[:, ht, ec * QC : (ec + 1) * QC],
                            start=(ht == 0),
                            stop=(ht == 1),
                        )
                    nc.vector.tensor_copy(ysb[:, ec * QC : (ec + 1) * QC], ps[:])
                nc.sync.dma_start(y.ap()[tt * P : (tt + 1) * P, :], ysb[:])

        # ---- interleaved schedule: per chunk c (ascending), project
        # K/V/Q for c then run attention(c); outproj trails by one chunk
        # to fill PE gaps under the exp chain. ----
        for c in range(NQC):
            proj_chunk(wk_sb, bk_sb, kT_sb, c)
            for kt in range(4 * c, 4 * c + 4):
                vproj_tile(kt)
            proj_chunk(wq_sb, bq_sb, qT_sb, c)
            for hp in range(2):
                attention(hp, c)
            if c > 0:
                outproj(c - 1)
        outproj(NQC - 1)


def make_core_inputs(x, w_qkv, b_qkv, w_out, b_out):
    """Shard + preprocess full inputs into 8 per-core input dicts."""
    bf16 = ml_dtypes.bfloat16
    x = np.asarray(x, np.float32)
    w_qkv = np.asarray(w_qkv, np.float32)
    b_qkv = np.asarray(b_qkv, np.float32)
    w_out = np.asarray(w_out, np.float32)

    # per-batch transpose+cast computed once and shared by the 4 cores
    xT_cache = [np.ascontiguousarray(x[b].T).astype(bf16) for b in range(B)]
    in_maps = []
    for c in range(N_CORES):
        b, g = divmod(c, 4)
        sl = slice(g * DH, (g + 1) * DH)
        wq = w_qkv[0 * D + g * DH : 0 * D + (g + 1) * DH]  # [DH, D]
        wk = w_qkv[1 * D + g * DH : 1 * D + (g + 1) * DH]
        wv = w_qkv[2 * D + g * DH : 2 * D + (g + 1) * DH]
        in_maps.append(
            {
                "xT": xT_cache[b],
                "wq": np.ascontiguousarray(wq.T).astype(bf16),
                "wk": np.ascontiguousarray(wk.T).astype(bf16),
                "wv": np.ascontiguousarray(wv.T).astype(bf16),
                "wo": np.ascontiguousarray(w_out[:, sl].T).astype(bf16),
                "bq": b_qkv[0 * D + g * DH : 0 * D + (g + 1) * DH]
                .reshape(2, P)
                .astype(np.float32),
                "bk": b_qkv[1 * D + g * DH : 1 * D + (g + 1) * DH]
                .reshape(2, P)
                .astype(np.float32),
                "bv": b_qkv[2 * D + g * DH : 2 * D + (g + 1) * DH]
                .reshape(1, DH)
                .astype(np.float32),
            }
        )
    return in_maps


def gather_output(results, b_out=None):
    """Sum the 4 per-core partials for each batch (+ b_out)."""
    out = np.empty((B, S, D), np.float32)
    for b in range(B):
        acc = results[4 * b]["y"].astype(np.float32)
        for g in range(1, 4):
            acc = acc + results[4 * b + g]["y"]
        out[b] = acc
    if b_out is not None:
        out += np.asarray(b_out, np.float32)
    return out


_NC_CACHE = None


def kernel(x, w_qkv, b_qkv, w_out, b_out):
    global _NC_CACHE
    from concourse.bass_utils import run_bass_kernel_spmd

    if _NC_CACHE is None:
        _NC_CACHE = build_nc()
    in_maps = make_core_inputs(x, w_qkv, b_qkv, w_out, b_out)
    res = run_bass_kernel_spmd(_NC_CACHE, in_maps, core_ids=list(range(N_CORES)))
    return gather_output(res.results, b_out=b_out)


# revision 3
# speedup vs baseline: 1.2135x; 1.2135x over previous
"""Trainium2 Bass kernel for causal self-attention (B=2, S=2048, D=1024, H=16).

Sharding: 8 cores = 2 batches x 4 head-groups (4 heads / 256 channels each).
Each core computes the qkv projection for its head block, causal attention for
its 4 heads, and a partial output projection (contraction over its 256
channels). The host sums the 4 partials per batch and adds b_out at gather.

Schedule: the kernel is ACT(exp)-bound during attention (~1147ns of exp per
k-tile vs ~640ns of PE work), while the QKV projection is pure PE work with
the ACT engine idle. Engines execute their instruction queues IN ORDER, so
overlap must be created at emission time: q-chunks run in ASCENDING order and
the projection for chunk c+1 plus the output projection for chunk c-1 are
emitted as small "quanta" (~4 matmuls) INSIDE the attention k-tile loop of
chunk c, filling the PE gaps under the exp chain.

Device dataflow (bf16 matmuls, fp32 accumulation):
  - x is fed pre-transposed (xT [D,S]) so every matmul has its contraction on
    partitions with no on-device transposes.
  - Q^T/K^T computed channel-major [ch, t]; V token-major [t, j] with a ones
    column appended -> the attention matmul produces numerator rows 0..63 and
    the softmax denominator in PSUM row 64 in one accumulation group.
  - scores computed transposed (k on partitions, q on free) so exp/mask/AV
    all chain without transposes; softmax needs no max-subtraction (scores
    are O(1) by construction) and division is deferred past the AV matmul.
  - causal: only k-tiles at/below the diagonal are computed; the 4 diagonal
    tiles per q-chunk restrict to their valid column range and apply a
    precomputed multiplicative 0/1 mask (on the Pool engine - DVE is loaded).
  - head pairs are packed into the 128-row PE array (base partitions 0/64,
    row-tiled 64x128 so the two heads' score matmuls run concurrently).
"""

import sys
from collections import deque

if "/opt/trn_rl_repo" not in sys.path:
    sys.path.insert(0, "/opt/trn_rl_repo")

import numpy as np
import ml_dtypes

import concourse.mybir as mybir
import concourse.tile as tile
from concourse import bacc

B, S, D, H, DK = 2, 2048, 1024, 16, 64
N_CORES = 8
HPC = 4  # heads per core
DH = HPC * DK  # 256 channels per core
P = 128
QC = 512  # q-chunk width
NQC = S // QC  # 4
NKT = S // P  # 16 k-tiles
DT = D // P  # 8 d-tiles
SCALE = 1.0 / np.sqrt(DK)

BF16 = mybir.dt.bfloat16
F32 = mybir.dt.float32


def build_nc(n_cores: int = N_CORES, repeats: int = 1):
    nc = bacc.Bacc("TRN2", target_bir_lowering=False, debug=False, num_devices=n_cores)

    xT = nc.dram_tensor("xT", [D, S], BF16, kind="ExternalInput")
    wq = nc.dram_tensor("wq", [D, DH], BF16, kind="ExternalInput")
    wk = nc.dram_tensor("wk", [D, DH], BF16, kind="ExternalInput")
    wv = nc.dram_tensor("wv", [D, DH], BF16, kind="ExternalInput")
    wo = nc.dram_tensor("wo", [DH, D], BF16, kind="ExternalInput")
    bq = nc.dram_tensor("bq", [2, P], F32, kind="ExternalInput")
    bk = nc.dram_tensor("bk", [2, P], F32, kind="ExternalInput")
    bv = nc.dram_tensor("bv", [1, DH], F32, kind="ExternalInput")
    y = nc.dram_tensor("y", [S, D], F32, kind="ExternalOutput")

    with tile.TileContext(nc) as tc:
        for _ in range(repeats):
            _body(nc, tc, xT, wq, wk, wv, wo, bq, bk, bv, y)

    nc.compile()
    return nc


def _body(nc, tc, xT, wq, wk, wv, wo, bq, bk, bv, y):
    add = mybir.AluOpType.add
    mult = mybir.AluOpType.mult
    Exp = mybir.ActivationFunctionType.Exp

    xT_r = xT.ap().rearrange("(dt p) t -> p dt t", p=P)
    wq_r = wq.ap().rearrange("(dt p) c -> p dt c", p=P)
    wk_r = wk.ap().rearrange("(dt p) c -> p dt c", p=P)
    wv_r = wv.ap().rearrange("(dt p) c -> p dt c", p=P)

    with (
        tc.tile_pool(name="const", bufs=1) as const,
        tc.tile_pool(name="work", bufs=6) as work,
        tc.tile_pool(name="psum", bufs=4, space="PSUM") as psum,
    ):
        # ---- persistent SBUF state (DMA split per chunk-slice so compute
        # can start as soon as the first slices land) ----
        xT_sb = const.tile([P, DT, S], BF16)
        wq_sb = const.tile([P, DT, DH], BF16)
        wk_sb = const.tile([P, DT, DH], BF16)
        wv_sb = const.tile([P, DT, DH], BF16)
        for dt in range(DT):
            nc.sync.dma_start(wk_sb[:, dt], wk_r[:, dt])
            nc.sync.dma_start(wq_sb[:, dt], wq_r[:, dt])
            nc.sync.dma_start(wv_sb[:, dt], wv_r[:, dt])
            nc.sync.dma_start(xT_sb[:, dt, 0:QC], xT_r[:, dt, 0:QC])
        for c in range(1, NQC):
            for dt in range(DT):
                nc.sync.dma_start(
                    xT_sb[:, dt, c * QC : (c + 1) * QC],
                    xT_r[:, dt, c * QC : (c + 1) * QC],
                )

        wo_sb = const.tile([P, 2, D], BF16)
        nc.sync.dma_start(wo_sb[:], wo.ap().rearrange("(ht p) e -> p ht e", p=P))

        bq_sb = const.tile([P, 2], F32)
        bk_sb = const.tile([P, 2], F32)
        nc.sync.dma_start(bq_sb[:], bq.ap().rearrange("mt p -> p mt"))
        nc.sync.dma_start(bk_sb[:], bk.ap().rearrange("mt p -> p mt"))

        bv_bc = const.tile([P, DH], F32)
        nc.sync.dma_start(bv_bc[0:1, :], bv.ap())
        nc.gpsimd.partition_broadcast(bv_bc[:], bv_bc[0:1, :])

        # causal masks for the 4 diagonal k-tiles of each q-chunk:
        # mask[p, i, ql] = 1.0 if p <= ql - 128*i else 0.0
        mask_sb = const.tile([P, 4, QC], BF16)
        nc.vector.memset(mask_sb[:], 1.0)
        for i in range(4):
            nc.gpsimd.affine_select(
                out=mask_sb[:, i, :],
                in_=mask_sb[:, i, :],
                compare_op=mybir.AluOpType.is_ge,
                fill=0.0,
                base=-P * i,
                pattern=[[1, QC]],
                channel_multiplier=-1,
            )

        qT_sb = const.tile([P, 2, S], BF16)  # [ch within mtile, mtile, t]
        kT_sb = const.tile([P, 2, S], BF16)
        # V' [t-part, ktile, head, dk+1]; col DK holds ones (softmax denom)
        vp_sb = const.tile([P, NKT, HPC, DK + 1], BF16)
        nc.vector.memset(vp_sb[:, :, :, DK : DK + 1], 1.0)
        aT_sb = const.tile([P, 2, S], BF16)  # attention out, channel-major

        # ---- filler quanta: ~4-matmul units of QKV / out-projection work
        # emitted inside the attention k-tile loop to fill PE gaps ----

        def proj_quanta(wsb, bsb, dst, c):
            # K^T/Q^T channel-major for chunk c: psum[ch,t] += w[d,ch].T@xT
            cs = slice(c * QC, (c + 1) * QC)
            out = []
            for mt in range(2):
                state = {}

                def q1(mt=mt, state=state):
                    state["ps"] = psum.tile([P, 2, QC], F32, tag="pair", name="pj")
                    for dt in range(4):
                        nc.tensor.matmul(
                            state["ps"][:, 0, :],
                            lhsT=wsb[:, dt, mt * P : (mt + 1) * P],
                            rhs=xT_sb[:, dt, cs],
                            start=(dt == 0),
                            stop=False,
                        )

                def q2(mt=mt, state=state):
                    ps = state["ps"]
                    for dt in range(4, DT):
                        nc.tensor.matmul(
                            ps[:, 0, :],
                            lhsT=wsb[:, dt, mt * P : (mt + 1) * P],
                            rhs=xT_sb[:, dt, cs],
                            start=False,
                            stop=(dt == DT - 1),
                        )
                    nc.vector.tensor_scalar(
                        dst[:, mt, cs], ps[:, 0, :], bsb[:, mt : mt + 1], None, op0=add
                    )

                out += [q1, q2]
            return out

        def vproj_quanta(c):
            # V token-major per k-tile: psum[t, j] += xT[d, t-tile].T @ wv
            out = []
            for kt in range(4 * c, 4 * c + 4):

                def q(kt=kt):
                    ps = psum.tile([P, 2, QC], F32, tag="pair", name="vps")
                    for dt in range(DT):
                        nc.tensor.matmul(
                            ps[:, 0, 0:DH],
                            lhsT=xT_sb[:, dt, kt * P : (kt + 1) * P],
                            rhs=wv_sb[:, dt, :],
                            start=(dt == 0),
                            stop=(dt == DT - 1),
                        )
                    nc.vector.tensor_tensor(
                        vp_sb[:, kt, :, 0:DK],
                        ps[:, 0, 0:DH].rearrange("p (h j) -> p h j", j=DK),
                        bv_bc[:].rearrange("p (h j) -> p h j", j=DK),
                        add,
                    )

                out.append(q)
            return out

        def outproj_quanta(qc):
            # partial y for t-tiles of chunk qc; b_out is added on the host
            out = []
            for tt in range(4 * qc, 4 * qc + 4):
                state = {}

                def q1(tt=tt, state=state):
                    state["ps"] = psum.tile([P, 2, QC], F32, tag="pair", name="yp")
                    for ht in range(2):
                        nc.tensor.matmul(
                            state["ps"][:, 0, :],
                            lhsT=aT_sb[:, ht, tt * P : (tt + 1) * P],
                            rhs=wo_sb[:, ht, 0:QC],
                            start=(ht == 0),
                            stop=(ht == 1),
                        )

                def q2(tt=tt, state=state):
                    ps = state["ps"]
                    for ht in range(2):
                        nc.tensor.matmul(
                            ps[:, 1, :],
                            lhsT=aT_sb[:, ht, tt * P : (tt + 1) * P],
                            rhs=wo_sb[:, ht, QC:D],
                            start=(ht == 0),
                            stop=(ht == 1),
                        )
                    ysb = work.tile([P, D], F32, tag="y")
                    nc.vector.tensor_copy(
                        ysb[:].rearrange("p (h q) -> p h q", h=2), ps[:]
                    )
                    nc.sync.dma_start(y.ap()[tt * P : (tt + 1) * P, :], ysb[:])

                out += [q1, q2]
            return out

        # ---- attention + lag-1 AV pipeline with filler injection ----
        def attention(hp, qc, fillers):
            nkt = 4 * (qc + 1)  # causal: k-tiles 0 .. 4*qc+3
            av = psum.tile([DK + 1, 2, QC], F32, tag="pair", name="av")
            pend = None  # delay AV by one k-tile to hide exp latency
            for kt in range(nkt):
                diag = kt - 4 * qc  # >= 0 on the 4 diagonal tiles
                cl = max(0, diag) * P  # first valid column of this q-chunk
                sc = psum.tile([P, 2, QC], F32, tag="pair", name="sc")
                for hh in range(2):
                    lo, hi = hh * DK, (hh + 1) * DK
                    nc.tensor.matmul(
                        sc[:, hh, cl:QC],
                        lhsT=kT_sb[lo:hi, hp, kt * P : (kt + 1) * P],
                        rhs=qT_sb[lo:hi, hp, qc * QC + cl : (qc + 1) * QC],
                        start=True,
                        stop=True,
                    )
                ex = work.tile([P, 2, QC], BF16, tag="exp", bufs=8)
                nc.scalar.activation(
                    ex[:, :, cl:QC], sc[:, :, cl:QC], Exp, scale=SCALE
                )
                if diag >= 0:
                    # causal mask on the Pool engine (SBUF-only op)
                    nc.gpsimd.tensor_tensor(
                        ex[:, :, cl:QC],
                        ex[:, :, cl:QC],
                        mask_sb[:, diag : diag + 1, cl:QC].to_broadcast(
                            (P, 2, QC - cl)
                        ),
                        mult,
                    )
                if pend is not None:
                    _av_pair(nc, av, vp_sb, hp, pend, qc, last=False)
                pend = (kt, ex)
                if fillers:
                    fillers.popleft()()
            _av_pair(nc, av, vp_sb, hp, pend, qc, last=True)

            # softmax division: row DK of av is the denominator; reciprocal
            # on partition DK, DMA the row to partition 0 (gpsimd broadcast
            # only honours base partition 0 on HW), broadcast down, multiply.
            qs = slice(qc * QC, (qc + 1) * QC)
            rec = work.tile([P, 2, QC], F32, tag="rec", bufs=2)
            nc.vector.reciprocal(rec[DK : DK + 1, :, :], av[DK : DK + 1, :, :])
            bcs = work.tile([1, 2, QC], F32, tag="bcs", bufs=2)
            nc.sync.dma_start(bcs[0:1, :, :], rec[DK : DK + 1, :, :])
            bc = work.tile([DK, 2, QC], F32, tag="bc")
            nc.gpsimd.partition_broadcast(bc[:], bcs[0:1, :, :])
            st = work.tile([DK, 2, QC], BF16, tag="st")
            nc.vector.tensor_mul(st[:], av[0:DK, :, :], bc[:])
            nc.sync.dma_start(aT_sb[0:DK, hp, qs], st[:, 0, :])
            nc.sync.dma_start(aT_sb[DK:P, hp, qs], st[:, 1, :])

        # ---- interleaved schedule: chunk 0's projection runs dense, then
        # each chunk's attention hides chunk c+1's projection and chunk
        # c-1's output projection as fillers. ----
        fillers = deque()
        for q in (
            proj_quanta(wk_sb, bk_sb, kT_sb, 0)
            + vproj_quanta(0)
            + proj_quanta(wq_sb, bq_sb, qT_sb, 0)
        ):
            q()  # chunk 0 projection: nothing to hide it under
        for c in range(NQC):
            if c + 1 < NQC:
                fillers.extend(proj_quanta(wk_sb, bk_sb, kT_sb, c + 1))
                fillers.extend(vproj_quanta(c + 1))
                fillers.extend(proj_quanta(wq_sb, bq_sb, qT_sb, c + 1))
            if c > 0:
                fillers.extend(outproj_quanta(c - 1))
            for hp in range(2):
                attention(hp, c, fillers)
            if c + 1 < NQC:
                while fillers:
                    fillers.popleft()()  # drain: chunk c+1 needs K/Q/V ready
        while fillers:
            fillers.popleft()()
        for q in outproj_quanta(NQC - 1):
            q()


def _av_pair(nc, av, vp_sb, hp, pend, qc, last):
    kt, ex = pend
    diag = kt - 4 * qc
    cl = max(0, diag) * P
    for hh in range(2):
        nc.tensor.matmul(
            av[:, hh, cl:QC],
            lhsT=vp_sb[:, kt, 2 * hp + hh, :],
            rhs=ex[:, hh, cl:QC],
            start=(kt == 0),
            stop=last,
        )


def make_core_inputs(x, w_qkv, b_qkv, w_out, b_out):
    """Shard + preprocess full inputs into 8 per-core input dicts."""
    bf16 = ml_dtypes.bfloat16
    x = np.asarray(x, np.float32)
    w_qkv = np.asarray(w_qkv, np.float32)
    b_qkv = np.asarray(b_qkv, np.float32)
    w_out = np.asarray(w_out, np.float32)

    # per-batch transpose+cast computed once and shared by the 4 cores
    xT_cache = [np.ascontiguousarray(x[b].T).astype(bf16) for b in range(B)]
    in_maps = []
    for c in range(N_CORES):
        b, g = divmod(c, 4)
        sl = slice(g * DH, (g + 1) * DH)
        wq = w_qkv[0 * D + g * DH : 0 * D + (g + 1) * DH]  # [DH, D]
        wk = w_qkv[1 * D + g * DH : 1 * D + (g + 1) * DH]
        wv = w_qkv[2 * D + g * DH : 2 * D + (g + 1) * DH]
        in_maps.append(
            {
                "xT": xT_cache[b],
                "wq": np.ascontiguousarray(wq.T).astype(bf16),
                "wk": np.ascontiguousarray(wk.T).astype(bf16),
                "wv": np.ascontiguousarray(wv.T).astype(bf16),
                "wo": np.ascontiguousarray(w_out[:, sl].T).astype(bf16),
                "bq": b_qkv[0 * D + g * DH : 0 * D + (g + 1) * DH]
                .reshape(2, P)
                .astype(np.float32),
                "bk": b_qkv[1 * D + g * DH : 1 * D + (g + 1) * DH]
                .reshape(2, P)
                .astype(np.float32),
                "bv": b_qkv[2 * D + g * DH : 2 * D + (g + 1) * DH]
                .reshape(1, DH)
                .astype(np.float32),
            }
        )
    return in_maps


def gather_output(results, b_out=None):
    """Sum the 4 per-core partials for each batch (+ b_out)."""
    out = np.empty((B, S, D), np.float32)
    for b in range(B):
        acc = results[4 * b]["y"].astype(np.float32)
        for g in range(1, 4):
            acc = acc + results[4 * b + g]["y"]
        out[b] = acc
    if b_out is not None:
        out += np.asarray(b_out, np.float32)
    return out


_NC_CACHE = None


def kernel(x, w_qkv, b_qkv, w_out, b_out):
    global _NC_CACHE
    from concourse.bass_utils import run_bass_kernel_spmd

    if _NC_CACHE is None:
        _NC_CACHE = build_nc()
    in_maps = make_core_inputs(x, w_qkv, b_qkv, w_out, b_out)
    res = run_bass_kernel_spmd(_NC_CACHE, in_maps, core_ids=list(range(N_CORES)))
    return gather_output(res.results, b_out=b_out)


# revision 18
# speedup vs baseline: 2.3162x; 1.9087x over previous
"""Trainium2 Bass kernel for causal self-attention (B=2, S=2048, D=1024, H=16).

Sharding: 8 cores = 2 batches x 4 head-groups (4 heads / 256 channels each).
Each core computes the qkv projection for its head block, causal attention for
its 4 heads, and a partial output projection (contraction over its 256
channels). The host sums the 4 partials per batch and adds b_out at gather.

Schedule: the kernel is ACT(exp)-bound during attention (~1147ns of exp per
k-tile vs ~640ns of PE work), while the QKV projection is pure PE work with
the ACT engine idle. Engines execute their instruction queues IN ORDER, so
overlap must be created at emission time: q-chunks run in ASCENDING order and
the projection for chunk c+1 plus the output projection for chunk c-1 are
emitted as small "quanta" (~4 matmuls) INSIDE the attention k-tile loop of
chunk c, filling the PE gaps under the exp chain.

Device dataflow (bf16 matmuls, fp32 accumulation):
  - x is fed pre-transposed (xT [D,S]) so every matmul has its contraction on
    partitions with no on-device transposes.
  - Q^T/K^T computed channel-major [ch, t]; V token-major [t, j] with a ones
    column appended -> the attention matmul produces numerator rows 0..63 and
    the softmax denominator in PSUM row 64 in one accumulation group.
  - scores computed transposed (k on partitions, q on free) so exp/mask/AV
    all chain without transposes; softmax needs no max-subtraction (scores
    are O(1) by construction) and division is deferred past the AV matmul.
  - causal: only k-tiles at/below the diagonal are computed; the 4 diagonal
    tiles per q-chunk restrict to their valid column range and apply a
    precomputed multiplicative 0/1 mask (on the Pool engine - DVE is loaded).
  - head pairs are packed into the 128-row PE array (base partitions 0/64,
    row-tiled 64x128 so the two heads' score matmuls run concurrently).
"""

import sys
from collections import deque

if "/opt/trn_rl_repo" not in sys.path:
    sys.path.insert(0, "/opt/trn_rl_repo")

import numpy as np
import ml_dtypes

import concourse.mybir as mybir
import concourse.tile as tile
from concourse import bacc

B, S, D, H, DK = 2, 2048, 1024, 16, 64
N_CORES = 8
HPC = 4  # heads per core
DH = HPC * DK  # 256 channels per core
P = 128
QC = 512  # q-chunk width
NQC = S // QC  # 4
NKT = S // P  # 16 k-tiles
DT = D // P  # 8 d-tiles
SCALE = 1.0 / np.sqrt(DK)

BF16 = mybir.dt.bfloat16
F32 = mybir.dt.float32


def build_nc(n_cores: int = N_CORES, repeats: int = 1):
    nc = bacc.Bacc("TRN2", target_bir_lowering=False, debug=False, num_devices=n_cores)

    xT = nc.dram_tensor("xT", [D, S], BF16, kind="ExternalInput")
    wq = nc.dram_tensor("wq", [D, DH], BF16, kind="ExternalInput")
    wk = nc.dram_tensor("wk", [D, DH], BF16, kind="ExternalInput")
    wv = nc.dram_tensor("wv", [D, DH], BF16, kind="ExternalInput")
    wo = nc.dram_tensor("wo", [DH, D], BF16, kind="ExternalInput")
    bq = nc.dram_tensor("bq", [2, P], F32, kind="ExternalInput")
    bk = nc.dram_tensor("bk", [2, P], F32, kind="ExternalInput")
    bv = nc.dram_tensor("bv", [1, DH], F32, kind="ExternalInput")
    y = nc.dram_tensor("y", [S, D], F32, kind="ExternalOutput")

    with tile.TileContext(nc) as tc:
        for _ in range(repeats):
            _body(nc, tc, xT, wq, wk, wv, wo, bq, bk, bv, y)

    nc.compile()
    return nc


def _body(nc, tc, xT, wq, wk, wv, wo, bq, bk, bv, y):
    add = mybir.AluOpType.add
    mult = mybir.AluOpType.mult
    Exp = mybir.ActivationFunctionType.Exp

    xT_r = xT.ap().rearrange("(dt p) t -> p dt t", p=P)
    wq_r = wq.ap().rearrange("(dt p) c -> p dt c", p=P)
    wk_r = wk.ap().rearrange("(dt p) c -> p dt c", p=P)
    wv_r = wv.ap().rearrange("(dt p) c -> p dt c", p=P)

    with (
        tc.tile_pool(name="const", bufs=1) as const,
        tc.tile_pool(name="work", bufs=6) as work,
        tc.tile_pool(name="psum", bufs=4, space="PSUM") as psum,
    ):
        # ---- persistent SBUF state (DMA split per chunk-slice so compute
        # can start as soon as the first slices land) ----
        xT_sb = const.tile([P, DT, S], BF16)
        wq_sb = const.tile([P, DT, DH], BF16)
        wk_sb = const.tile([P, DT, DH], BF16)
        wv_sb = const.tile([P, DT, DH], BF16)
        wo_sb = const.tile([P, 2, D], BF16)
        bq_sb = const.tile([P, 2], F32)
        bk_sb = const.tile([P, 2], F32)
        bv_bc = const.tile([P, DH], F32)

        # DMA placement: SP carries the latency-critical input path (biases
        # first, then wk/wv/xT-chunk0 interleaved in first-use order); wq
        # rides the ACT hwdge queue in parallel (ACT is idle before the
        # first exp); xT chunks 2-3 are emitted later, behind the first
        # attention's division DMAs, so those never queue behind bulk.
        nc.sync.dma_start(bk_sb[:], bk.ap().rearrange("mt p -> p mt"))
        nc.sync.dma_start(bq_sb[:], bq.ap().rearrange("mt p -> p mt"))
        nc.sync.dma_start(bv_bc[0:1, :], bv.ap())
        nc.gpsimd.partition_broadcast(bv_bc[:], bv_bc[0:1, :])
        for dt in range(DT):
            nc.scalar.dma_start(wq_sb[:, dt], wq_r[:, dt])
        for dt in range(DT):
            nc.scalar.dma_start(wv_sb[:, dt], wv_r[:, dt])
        for dt in range(DT):
            nc.sync.dma_start(wk_sb[:, dt], wk_r[:, dt])
            nc.sync.dma_start(xT_sb[:, dt, 0:QC], xT_r[:, dt, 0:QC])
        for dt in range(DT):
            nc.sync.dma_start(xT_sb[:, dt, QC : 2 * QC], xT_r[:, dt, QC : 2 * QC])
        nc.sync.dma_start(wo_sb[:], wo.ap().rearrange("(ht p) e -> p ht e", p=P))

        def xt_chunk_dma(c):
            for dt in range(DT):
                nc.sync.dma_start(
                    xT_sb[:, dt, c * QC : (c + 1) * QC],
                    xT_r[:, dt, c * QC : (c + 1) * QC],
                )

        # causal masks for the 4 diagonal k-tiles of each q-chunk:
        # mask[p, i, ql] = 1.0 if p <= ql - 128*i else 0.0
        mask_sb = const.tile([P, 4, QC], BF16)
        nc.vector.memset(mask_sb[:], 1.0)
        for i in range(4):
            nc.gpsimd.affine_select(
                out=mask_sb[:, i, :],
                in_=mask_sb[:, i, :],
                compare_op=mybir.AluOpType.is_ge,
                fill=0.0,
                base=-P * i,
                pattern=[[1, QC]],
                channel_multiplier=-1,
            )

        qT_sb = const.tile([P, 2, S], BF16)  # [ch within mtile, mtile, t]
        kT_sb = const.tile([P, 2, S], BF16)
        # V' [t-part, ktile, head, dk+1]; col DK holds ones (softmax denom)
        vp_sb = const.tile([P, NKT, HPC, DK + 1], BF16)
        nc.vector.memset(vp_sb[:, :, :, DK : DK + 1], 1.0)
        aT_sb = const.tile([P, 2, S], BF16)  # attention out, channel-major

        # ---- filler quanta: ~4-matmul units of QKV / out-projection work
        # emitted inside the attention k-tile loop to fill PE gaps ----

        def proj_quanta(wsb, bsb, dst, c, mt):
            # K^T/Q^T channel-major for chunk c, head-pair mt:
            # psum[ch, t] += w[d, ch].T @ xT[d, t]  (two 4-matmul quanta)
            cs = slice(c * QC, (c + 1) * QC)
            state = {}

            def q1():
                state["ps"] = psum.tile([P, 2, QC], F32, tag="pair", name="pj")
                for dt in range(4):
                    nc.tensor.matmul(
                        state["ps"][:, 0, :],
                        lhsT=wsb[:, dt, mt * P : (mt + 1) * P],
                        rhs=xT_sb[:, dt, cs],
                        start=(dt == 0),
                        stop=False,
                    )

            def q2():
                ps = state["ps"]
                for dt in range(4, DT):
                    nc.tensor.matmul(
                        ps[:, 0, :],
                        lhsT=wsb[:, dt, mt * P : (mt + 1) * P],
                        rhs=xT_sb[:, dt, cs],
                        start=False,
                        stop=(dt == DT - 1),
                    )
                nc.vector.tensor_scalar(
                    dst[:, mt, cs], ps[:, 0, :], bsb[:, mt : mt + 1], None, op0=add
                )

            return [q1, q2]

        def kq_quanta(c, mt):
            return proj_quanta(wk_sb, bk_sb, kT_sb, c, mt) + proj_quanta(
                wq_sb, bq_sb, qT_sb, c, mt
            )

        def vproj_quanta(c):
            # V token-major per k-tile: psum[t, j] += xT[d, t-tile].T @ wv
            out = []
            for kt in range(4 * c, 4 * c + 4):

                def q(kt=kt):
                    ps = psum.tile([P, 2, QC], F32, tag="pair", name="vps")
                    for dt in range(DT):
                        nc.tensor.matmul(
                            ps[:, 0, 0:DH],
                            lhsT=xT_sb[:, dt, kt * P : (kt + 1) * P],
                            rhs=wv_sb[:, dt, :],
                            start=(dt == 0),
                            stop=(dt == DT - 1),
                        )
                    nc.vector.tensor_tensor(
                        vp_sb[:, kt, :, 0:DK],
                        ps[:, 0, 0:DH].rearrange("p (h j) -> p h j", j=DK),
                        bv_bc[:].rearrange("p (h j) -> p h j", j=DK),
                        add,
                    )

                out.append(q)
            return out

        def outproj_quanta(qc):
            # partial y for t-tiles of chunk qc; b_out is added on the host.
            # y stores for early chunks ride the gpsimd DGE queue (keeps SP
            # free for the next repeat's inputs); the last chunk's stores go
            # on SP, which is idle by then, so they don't delay the
            # division broadcasts that share the Pool queue at the tail.
            out = []
            for tt in range(4 * qc, 4 * qc + 4):
                state = {}

                def q1(tt=tt, state=state):
                    state["ps"] = psum.tile([P, 2, QC], F32, tag="pair", name="yp")
                    for ht in range(2):
                        nc.tensor.matmul(
                            state["ps"][:, 0, :],
                            lhsT=aT_sb[:, ht, tt * P : (tt + 1) * P],
                            rhs=wo_sb[:, ht, 0:QC],
                            start=(ht == 0),
                            stop=(ht == 1),
                        )

                def q2(tt=tt, state=state, qc=qc):
                    ps = state["ps"]
                    for ht in range(2):
                        nc.tensor.matmul(
                            ps[:, 1, :],
                            lhsT=aT_sb[:, ht, tt * P : (tt + 1) * P],
                            rhs=wo_sb[:, ht, QC:D],
                            start=(ht == 0),
                            stop=(ht == 1),
                        )
                    ysb = work.tile([P, D], F32, tag="y")
                    nc.vector.tensor_copy(
                        ysb[:].rearrange("p (h q) -> p h q", h=2), ps[:]
                    )
                    nc.sync.dma_start(y.ap()[tt * P : (tt + 1) * P, :], ysb[:])

                out += [q1, q2]
            return out

        # ---- attention + lag-1 AV pipeline with filler injection ----
        def attention(hp, qc, fillers):
            nkt = 4 * (qc + 1)  # causal: k-tiles 0 .. 4*qc+3
            av = psum.tile([DK + 1, 2, QC], F32, tag="pair", name="av")
            pend = None  # delay AV by one k-tile to hide exp latency
            for kt in range(nkt):
                diag = kt - 4 * qc  # >= 0 on the 4 diagonal tiles
                cl = max(0, diag) * P  # first valid column of this q-chunk
                sc = psum.tile([P, 2, QC], F32, tag="pair", name="sc")
                for hh in range(2):
                    lo, hi = hh * DK, (hh + 1) * DK
                    nc.tensor.matmul(
                        sc[:, hh, cl:QC],
                        lhsT=kT_sb[lo:hi, hp, kt * P : (kt + 1) * P],
                        rhs=qT_sb[lo:hi, hp, qc * QC + cl : (qc + 1) * QC],
                        start=True,
                        stop=True,
                    )
                ex = work.tile([P, 2, QC], BF16, tag="exp", bufs=8)
                nc.scalar.activation(
                    ex[:, :, cl:QC], sc[:, :, cl:QC], Exp, scale=SCALE
                )
                if diag >= 0:
                    nc.vector.tensor_mul(
                        ex[:, :, cl:QC],
                        ex[:, :, cl:QC],
                        mask_sb[:, diag : diag + 1, cl:QC].to_broadcast(
                            (P, 2, QC - cl)
                        ),
                    )
                if pend is not None:
                    _av_pair(nc, av, vp_sb, hp, pend, qc, last=False)
                pend = (kt, ex)
                if fillers:
                    fillers.popleft()()
            _av_pair(nc, av, vp_sb, hp, pend, qc, last=True)

            # softmax division: row DK of av is the denominator; reciprocal
            # on partition DK, DMA the row to partition 0 (gpsimd broadcast
            # only honours base partition 0 on HW), broadcast down, multiply.
            qs = slice(qc * QC, (qc + 1) * QC)
            rec = work.tile([P, 2, QC], F32, tag="rec", bufs=2)
            nc.vector.reciprocal(rec[DK : DK + 1, :, :], av[DK : DK + 1, :, :])
            bcs = work.tile([1, 2, QC], F32, tag="bcs", bufs=2)
            nc.sync.dma_start(bcs[0:1, :, :], rec[DK : DK + 1, :, :])
            bc = work.tile([DK, 2, QC], F32, tag="bc")
            nc.gpsimd.partition_broadcast(bc[:], bcs[0:1, :, :])
            st = work.tile([DK, 2, QC], BF16, tag="st")
            nc.vector.tensor_mul(st[:], av[0:DK, :, :], bc[:])
            nc.sync.dma_start(aT_sb[0:DK, hp, qs], st[:, 0, :])
            nc.sync.dma_start(aT_sb[DK:P, hp, qs], st[:, 1, :])

        # ---- interleaved schedule (ascending chunks, zero dense drains):
        # attention(hp0, c) consumes [V(c), K/Q-mt1(c)] as fillers (mt = hp,
        # so hp0 only needs the mt0 projections up front); attention(hp1, c)
        # consumes [K/Q-mt0(c+1), outproj(c-1)]. Pop counts verified to fit
        # each attention's k-tile step count with dependencies in order. ----
        fillers = deque()
        for q in kq_quanta(0, 0) + vproj_quanta(0):
            q()  # minimal dense head: K/Q mt0 + V of chunk 0
        for c in range(NQC):
            if c > 0:
                fillers.extend(vproj_quanta(c))
            fillers.extend(kq_quanta(c, 1))
            attention(0, c, fillers)
            if c == 0:
                xt_chunk_dma(2)  # behind attention(0,0)'s division DMAs
            if c + 1 < NQC:
                fillers.extend(kq_quanta(c + 1, 0))
            if c > 0:
                fillers.extend(outproj_quanta(c - 1))
            attention(1, c, fillers)
            if c == 0:
                xt_chunk_dma(3)
        while fillers:
            fillers.popleft()()
        for q in outproj_quanta(NQC - 1):
            q()


def _av_pair(nc, av, vp_sb, hp, pend, qc, last):
    kt, ex = pend
    diag = kt - 4 * qc
    cl = max(0, diag) * P
    for hh in range(2):
        nc.tensor.matmul(
            av[:, hh, cl:QC],
            lhsT=vp_sb[:, kt, 2 * hp + hh, :],
            rhs=ex[:, hh, cl:QC],
            start=(kt == 0),
            stop=last,
        )


def make_core_inputs(x, w_qkv, b_qkv, w_out, b_out):
    """Shard + preprocess full inputs into 8 per-core input dicts."""
    bf16 = ml_dtypes.bfloat16
    x = np.asarray(x, np.float32)
    w_qkv = np.asarray(w_qkv, np.float32)
    b_qkv = np.asarray(b_qkv, np.float32)
    w_out = np.asarray(w_out, np.float32)

    # per-batch transpose+cast computed once and shared by the 4 cores
    xT_cache = [np.ascontiguousarray(x[b].T).astype(bf16) for b in range(B)]
    in_maps = []
    for c in range(N_CORES):
        b, g = divmod(c, 4)
        sl = slice(g * DH, (g + 1) * DH)
        wq = w_qkv[0 * D + g * DH : 0 * D + (g + 1) * DH]  # [DH, D]
        wk = w_qkv[1 * D + g * DH : 1 * D + (g + 1) * DH]
        wv = w_qkv[2 * D + g * DH : 2 * D + (g + 1) * DH]
        in_maps.append(
            {
                "xT": xT_cache[b],
                "wq": np.ascontiguousarray(wq.T).astype(bf16),
                "wk": np.ascontiguousarray(wk.T).astype(bf16),
                "wv": np.ascontiguousarray(wv.T).astype(bf16),
                "wo": np.ascontiguousarray(w_out[:, sl].T).astype(bf16),
                "bq": b_qkv[0 * D + g * DH : 0 * D + (g + 1) * DH]
                .reshape(2, P)
                .astype(np.float32),
                "bk": b_qkv[1 * D + g * DH : 1 * D + (g + 1) * DH]
                .reshape(2, P)
                .astype(np.float32),
                "bv": b_qkv[2 * D + g * DH : 2 * D + (g + 1) * DH]
                .reshape(1, DH)
                .astype(np.float32),
            }
        )
    return in_maps


def gather_output(results, b_out=None):
    """Sum the 4 per-core partials for each batch (+ b_out)."""
    out = np.empty((B, S, D), np.float32)
    for b in range(B):
        acc = results[4 * b]["y"].astype(np.float32)
        for g in range(1, 4):
            acc = acc + results[4 * b + g]["y"]
        out[b] = acc
    if b_out is not None:
        out += np.asarray(b_out, np.float32)
    return out


_NC_CACHE = None


def kernel(x, w_qkv, b_qkv, w_out, b_out):
    global _NC_CACHE
    from concourse.bass_utils import run_bass_kernel_spmd

    if _NC_CACHE is None:
        _NC_CACHE = build_nc()
    in_maps = make_core_inputs(x, w_qkv, b_qkv, w_out, b_out)
    res = run_bass_kernel_spmd(_NC_CACHE, in_maps, core_ids=list(range(N_CORES)))
    return gather_output(res.results, b_out=b_out)


# revision 22
# speedup vs baseline: 2.5113x; 1.0842x over previous
"""Trainium2 Bass kernel for causal self-attention (B=2, S=2048, D=1024, H=16).

Sharding: 8 cores = 2 batches x 4 head-groups (4 heads / 256 channels each).
Each core computes the qkv projection for its head block, causal attention for
its 4 heads, and a partial output projection (contraction over its 256
channels). The host sums the 4 partials per batch and adds b_out at gather.

Schedule: the kernel is ACT(exp)-bound during attention (~1147ns of exp per
k-tile vs ~640ns of PE work), while the QKV projection is pure PE work with
the ACT engine idle. Engines execute their instruction queues IN ORDER, so
overlap must be created at emission time: q-chunks run in ASCENDING order and
the projection for chunk c+1 plus the output projection for chunk c-1 are
emitted as small "quanta" (~4 matmuls) INSIDE the attention k-tile loop of
chunk c, filling the PE gaps under the exp chain.

Device dataflow (bf16 matmuls, fp32 accumulation):
  - x is fed pre-transposed (xT [D,S]) so every matmul has its contraction on
    partitions with no on-device transposes.
  - Q^T/K^T computed channel-major [ch, t]; V token-major [t, j] with a ones
    column appended -> the attention matmul produces numerator rows 0..63 and
    the softmax denominator in PSUM row 64 in one accumulation group.
  - scores computed transposed (k on partitions, q on free) so exp/mask/AV
    all chain without transposes; softmax needs no max-subtraction (scores
    are O(1) by construction) and division is deferred past the AV matmul.
  - causal: only k-tiles at/below the diagonal are computed; the 4 diagonal
    tiles per q-chunk restrict to their valid column range and apply a
    precomputed multiplicative 0/1 mask (on the Pool engine - DVE is loaded).
  - head pairs are packed into the 128-row PE array (base partitions 0/64,
    row-tiled 64x128 so the two heads' score matmuls run concurrently).
"""

import sys
from collections import deque

if "/opt/trn_rl_repo" not in sys.path:
    sys.path.insert(0, "/opt/trn_rl_repo")

import numpy as np
import ml_dtypes

import concourse.mybir as mybir
import concourse.tile as tile
from concourse import bacc

B, S, D, H, DK = 2, 2048, 1024, 16, 64
N_CORES = 8
HPC = 4  # heads per core
DH = HPC * DK  # 256 channels per core
P = 128
QC = 512  # q-chunk width
NQC = S // QC  # 4
NKT = S // P  # 16 k-tiles
DT = D // P  # 8 d-tiles
SCALE = 1.0 / np.sqrt(DK)

BF16 = mybir.dt.bfloat16
F32 = mybir.dt.float32


def build_nc(n_cores: int = N_CORES, repeats: int = 1):
    nc = bacc.Bacc("TRN2", target_bir_lowering=False, debug=False, num_devices=n_cores)

    xT = nc.dram_tensor("xT", [D, S], BF16, kind="ExternalInput")
    wq = nc.dram_tensor("wq", [D, DH], BF16, kind="ExternalInput")
    wk = nc.dram_tensor("wk", [D, DH], BF16, kind="ExternalInput")
    wv = nc.dram_tensor("wv", [D, DH], BF16, kind="ExternalInput")
    wo = nc.dram_tensor("wo", [DH, D], BF16, kind="ExternalInput")
    bq = nc.dram_tensor("bq", [2, P], F32, kind="ExternalInput")
    bk = nc.dram_tensor("bk", [2, P], F32, kind="ExternalInput")
    bv = nc.dram_tensor("bv", [1, DH], F32, kind="ExternalInput")
    y = nc.dram_tensor("y", [S, D], F32, kind="ExternalOutput")

    with tile.TileContext(nc) as tc:
        for _ in range(repeats):
            _body(nc, tc, xT, wq, wk, wv, wo, bq, bk, bv, y)

    nc.compile()
    return nc


def _body(nc, tc, xT, wq, wk, wv, wo, bq, bk, bv, y):
    add = mybir.AluOpType.add
    mult = mybir.AluOpType.mult
    Exp = mybir.ActivationFunctionType.Exp

    xT_r = xT.ap().rearrange("(dt p) t -> p dt t", p=P)
    wq_r = wq.ap().rearrange("(dt p) c -> p dt c", p=P)
    wk_r = wk.ap().rearrange("(dt p) c -> p dt c", p=P)
    wv_r = wv.ap().rearrange("(dt p) c -> p dt c", p=P)

    with (
        tc.tile_pool(name="const", bufs=1) as const,
        tc.tile_pool(name="work", bufs=6) as work,
        tc.tile_pool(name="psum", bufs=4, space="PSUM") as psum,
    ):
        # ---- persistent SBUF state (DMA split per chunk-slice so compute
        # can start as soon as the first slices land) ----
        xT_sb = const.tile([P, DT, S], BF16)
        wq_sb = const.tile([P, DT, DH], BF16)
        wk_sb = const.tile([P, DT, DH], BF16)
        wv_sb = const.tile([P, DT, DH], BF16)
        wo_sb = const.tile([P, 2, D], BF16)
        bq_sb = const.tile([P, 2], F32)
        bk_sb = const.tile([P, 2], F32)
        bv_bc = const.tile([P, DH], F32)

        # DMA placement: SP carries the latency-critical input path (biases
        # first, then wk/wv/xT-chunk0 interleaved in first-use order); wq
        # rides the ACT hwdge queue in parallel (ACT is idle before the
        # first exp); xT chunks 2-3 are emitted later, behind the first
        # attention's division DMAs, so those never queue behind bulk.
        nc.sync.dma_start(bk_sb[:], bk.ap().rearrange("mt p -> p mt"))
        nc.sync.dma_start(bq_sb[:], bq.ap().rearrange("mt p -> p mt"))
        nc.sync.dma_start(bv_bc[0:1, :], bv.ap())
        nc.gpsimd.partition_broadcast(bv_bc[:], bv_bc[0:1, :])
        for dt in range(DT):
            nc.scalar.dma_start(wq_sb[:, dt], wq_r[:, dt])
        for dt in range(DT):
            nc.scalar.dma_start(wv_sb[:, dt], wv_r[:, dt])
        for dt in range(DT):
            nc.sync.dma_start(wk_sb[:, dt], wk_r[:, dt])
            nc.sync.dma_start(xT_sb[:, dt, 0:QC], xT_r[:, dt, 0:QC])
        for dt in range(DT):
            nc.sync.dma_start(xT_sb[:, dt, QC : 2 * QC], xT_r[:, dt, QC : 2 * QC])
        nc.sync.dma_start(wo_sb[:], wo.ap().rearrange("(ht p) e -> p ht e", p=P))

        def xt_chunk_dma(c):
            for dt in range(DT):
                nc.sync.dma_start(
                    xT_sb[:, dt, c * QC : (c + 1) * QC],
                    xT_r[:, dt, c * QC : (c + 1) * QC],
                )

        # causal masks for the 4 diagonal k-tiles of each q-chunk:
        # mask[p, i, ql] = 1.0 if p <= ql - 128*i else 0.0
        mask_sb = const.tile([P, 4, QC], BF16)
        nc.vector.memset(mask_sb[:], 1.0)
        for i in range(4):
            nc.gpsimd.affine_select(
                out=mask_sb[:, i, :],
                in_=mask_sb[:, i, :],
                compare_op=mybir.AluOpType.is_ge,
                fill=0.0,
                base=-P * i,
                pattern=[[1, QC]],
                channel_multiplier=-1,
            )

        qT_sb = const.tile([P, 2, S], BF16)  # [ch within mtile, mtile, t]
        kT_sb = const.tile([P, 2, S], BF16)
        # V' [t-part, ktile, head, dk+1]; col DK holds ones (softmax denom)
        vp_sb = const.tile([P, NKT, HPC, DK + 1], BF16)
        nc.vector.memset(vp_sb[:, :, :, DK : DK + 1], 1.0)
        aT_sb = const.tile([P, 2, S], BF16)  # attention out, channel-major

        # ---- filler quanta: ~4-matmul units of QKV / out-projection work
        # emitted inside the attention k-tile loop to fill PE gaps ----

        def proj_quanta(wsb, bsb, dst, c, mt):
            # K^T/Q^T channel-major for chunk c, head-pair mt:
            # psum[ch, t] += w[d, ch].T @ xT[d, t]  (two 4-matmul quanta)
            cs = slice(c * QC, (c + 1) * QC)
            state = {}

            def q1():
                state["ps"] = psum.tile([P, 2, QC], F32, tag="pair", name="pj")
                for dt in range(4):
                    nc.tensor.matmul(
                        state["ps"][:, 0, :],
                        lhsT=wsb[:, dt, mt * P : (mt + 1) * P],
                        rhs=xT_sb[:, dt, cs],
                        start=(dt == 0),
                        stop=False,
                    )

            def q2():
                ps = state["ps"]
                for dt in range(4, DT):
                    nc.tensor.matmul(
                        ps[:, 0, :],
                        lhsT=wsb[:, dt, mt * P : (mt + 1) * P],
                        rhs=xT_sb[:, dt, cs],
                        start=False,
                        stop=(dt == DT - 1),
                    )
                nc.vector.tensor_scalar(
                    dst[:, mt, cs], ps[:, 0, :], bsb[:, mt : mt + 1], None, op0=add
                )

            return [q1, q2]

        def kq_quanta(c, mt):
            return proj_quanta(wk_sb, bk_sb, kT_sb, c, mt) + proj_quanta(
                wq_sb, bq_sb, qT_sb, c, mt
            )

        def vproj_quanta(c):
            # V token-major per k-tile: psum[t, j] += xT[d, t-tile].T @ wv
            out = []
            for kt in range(4 * c, 4 * c + 4):

                def q(kt=kt):
                    ps = psum.tile([P, 2, QC], F32, tag="pair", name="vps")
                    for dt in range(DT):
                        nc.tensor.matmul(
                            ps[:, 0, 0:DH],
                            lhsT=xT_sb[:, dt, kt * P : (kt + 1) * P],
                            rhs=wv_sb[:, dt, :],
                            start=(dt == 0),
                            stop=(dt == DT - 1),
                        )
                    nc.vector.tensor_tensor(
                        vp_sb[:, kt, :, 0:DK],
                        ps[:, 0, 0:DH].rearrange("p (h j) -> p h j", j=DK),
                        bv_bc[:].rearrange("p (h j) -> p h j", j=DK),
                        add,
                    )

                out.append(q)
            return out

        def outproj_quanta(qc):
            # partial y for t-tiles of chunk qc; b_out is added on the host.
            # y stores for early chunks ride the gpsimd DGE queue (keeps SP
            # free for the next repeat's inputs); the last chunk's stores go
            # on SP, which is idle by then, so they don't delay the
            # division broadcasts that share the Pool queue at the tail.
            out = []
            for tt in range(4 * qc, 4 * qc + 4):
                state = {}

                def q1(tt=tt, state=state):
                    state["ps"] = psum.tile([P, 2, QC], F32, tag="pair", name="yp")
                    for ht in range(2):
                        nc.tensor.matmul(
                            state["ps"][:, 0, :],
                            lhsT=aT_sb[:, ht, tt * P : (tt + 1) * P],
                            rhs=wo_sb[:, ht, 0:QC],
                            start=(ht == 0),
                            stop=(ht == 1),
                        )

                def q2(tt=tt, state=state, qc=qc):
                    ps = state["ps"]
                    for ht in range(2):
                        nc.tensor.matmul(
                            ps[:, 1, :],
                            lhsT=aT_sb[:, ht, tt * P : (tt + 1) * P],
                            rhs=wo_sb[:, ht, QC:D],
                            start=(ht == 0),
                            stop=(ht == 1),
                        )
                    ysb = work.tile([P, D], F32, tag="y")
                    nc.vector.tensor_copy(
                        ysb[:].rearrange("p (h q) -> p h q", h=2), ps[:]
                    )
                    nc.sync.dma_start(y.ap()[tt * P : (tt + 1) * P, :], ysb[:])

                out += [q1, q2]
            return out

        # ---- attention + lag-1 AV pipeline with filler injection ----
        def attention(hp, qc, fillers):
            nkt = 4 * (qc + 1)  # causal: k-tiles 0 .. 4*qc+3
            av = psum.tile([DK + 1, 2, QC], F32, tag="pair", name="av")
            pend = None  # delay AV by one k-tile to hide exp latency
            for kt in range(nkt):
                diag = kt - 4 * qc  # >= 0 on the 4 diagonal tiles
                cl = max(0, diag) * P  # first valid column of this q-chunk
                sc = psum.tile([P, 2, QC], F32, tag="pair", name="sc")
                for hh in range(2):
                    lo, hi = hh * DK, (hh + 1) * DK
                    nc.tensor.matmul(
                        sc[:, hh, cl:QC],
                        lhsT=kT_sb[lo:hi, hp, kt * P : (kt + 1) * P],
                        rhs=qT_sb[lo:hi, hp, qc * QC + cl : (qc + 1) * QC],
                        start=True,
                        stop=True,
                    )
                ex = work.tile([P, 2, QC], BF16, tag="exp", bufs=8)
                nc.scalar.activation(
                    ex[:, :, cl:QC], sc[:, :, cl:QC], Exp, scale=SCALE
                )
                if diag >= 0:
                    nc.vector.tensor_mul(
                        ex[:, :, cl:QC],
                        ex[:, :, cl:QC],
                        mask_sb[:, diag : diag + 1, cl:QC].to_broadcast(
                            (P, 2, QC - cl)
                        ),
                    )
                if pend is not None:
                    _av_pair(nc, av, vp_sb, hp, pend, qc, last=False)
                pend = (kt, ex)
                if fillers:
                    fillers.popleft()()
            _av_pair(nc, av, vp_sb, hp, pend, qc, last=True)

            # softmax division: row DK of av is the denominator; reciprocal
            # on partition DK, DMA the row to partition 0 (gpsimd broadcast
            # only honours base partition 0 on HW), broadcast down, multiply.
            qs = slice(qc * QC, (qc + 1) * QC)
            rec = work.tile([P, 2, QC], F32, tag="rec", bufs=2)
            nc.vector.reciprocal(rec[DK : DK + 1, :, :], av[DK : DK + 1, :, :])
            bcs = work.tile([1, 2, QC], F32, tag="bcs", bufs=2)
            nc.sync.dma_start(bcs[0:1, :, :], rec[DK : DK + 1, :, :])
            bc = work.tile([DK, 2, QC], F32, tag="bc")
            nc.gpsimd.partition_broadcast(bc[:], bcs[0:1, :, :])
            st = work.tile([DK, 2, QC], BF16, tag="st")
            nc.vector.tensor_mul(st[:], av[0:DK, :, :], bc[:])
            nc.sync.dma_start(aT_sb[0:DK, hp, qs], st[:, 0, :])
            nc.sync.dma_start(aT_sb[DK:P, hp, qs], st[:, 1, :])

        # ---- interleaved schedule (ascending chunks, zero dense drains):
        # attention(hp0, c) consumes [V(c), K/Q-mt1(c)] as fillers (mt = hp,
        # so hp0 only needs the mt0 projections up front); attention(hp1, c)
        # consumes [K/Q-mt0(c+1), outproj(c-1)]. Pop counts verified to fit
        # each attention's k-tile step count with dependencies in order. ----
        fillers = deque()
        for q in kq_quanta(0, 0) + vproj_quanta(0):
            q()  # minimal dense head: K/Q mt0 + V of chunk 0
        for c in range(NQC):
            if c > 0:
                fillers.extend(vproj_quanta(c))
            fillers.extend(kq_quanta(c, 1))
            attention(0, c, fillers)
            if c == 0:
                xt_chunk_dma(2)  # behind attention(0,0)'s division DMAs
            if c + 1 < NQC:
                fillers.extend(kq_quanta(c + 1, 0))
            if c > 0:
                fillers.extend(outproj_quanta(c - 1))
            attention(1, c, fillers)
            if c == 0:
                xt_chunk_dma(3)
        while fillers:
            fillers.popleft()()
        for q in outproj_quanta(NQC - 1):
            q()


def _av_pair(nc, av, vp_sb, hp, pend, qc, last):
    kt, ex = pend
    diag = kt - 4 * qc
    cl = max(0, diag) * P
    for hh in range(2):
        nc.tensor.matmul(
            av[:, hh, cl:QC],
            lhsT=vp_sb[:, kt, 2 * hp + hh, :],
            rhs=ex[:, hh, cl:QC],
            start=(kt == 0),
            stop=last,
        )


def make_core_inputs(x, w_qkv, b_qkv, w_out, b_out):
    """Shard + preprocess full inputs into 8 per-core input dicts."""
    bf16 = ml_dtypes.bfloat16
    x = np.asarray(x, np.float32)
    w_qkv = np.asarray(w_qkv, np.float32)
    b_qkv = np.asarray(b_qkv, np.float32)
    w_out = np.asarray(w_out, np.float32)

    # per-batch transpose+cast computed once and shared by the 4 cores
    xT_cache = [np.ascontiguousarray(x[b].T).astype(bf16) for b in range(B)]
    in_maps = []
    for c in range(N_CORES):
        b, g = divmod(c, 4)
        sl = slice(g * DH, (g + 1) * DH)
        wq = w_qkv[0 * D + g * DH : 0 * D + (g + 1) * DH]  # [DH, D]
        wk = w_qkv[1 * D + g * DH : 1 * D + (g + 1) * DH]
        wv = w_qkv[2 * D + g * DH : 2 * D + (g + 1) * DH]
        in_maps.append(
            {
                "xT": xT_cache[b],
                "wq": np.ascontiguousarray(wq.T).astype(bf16),
                "wk": np.ascontiguousarray(wk.T).astype(bf16),
                "wv": np.ascontiguousarray(wv.T).astype(bf16),
                "wo": np.ascontiguousarray(w_out[:, sl].T).astype(bf16),
                "bq": b_qkv[0 * D + g * DH : 0 * D + (g + 1) * DH]
                .reshape(2, P)
                .astype(np.float32),
                "bk": b_qkv[1 * D + g * DH : 1 * D + (g + 1) * DH]
                .reshape(2, P)
                .astype(np.float32),
                "bv": b_qkv[2 * D + g * DH : 2 * D + (g + 1) * DH]
                .reshape(1, DH)
                .astype(np.float32),
            }
        )
    return in_maps


def gather_output(results, b_out=None):
    """Sum the 4 per-core partials for each batch (+ b_out)."""
    out = np.empty((B, S, D), np.float32)
    for b in range(B):
        acc = results[4 * b]["y"].astype(np.float32)
        for g in range(1, 4):
            acc = acc + results[4 * b + g]["y"]
        out[b] = acc
    if b_out is not None:
        out += np.asarray(b_out, np.float32)
    return out


_NC_CACHE = None


def kernel(x, w_qkv, b_qkv, w_out, b_out):
    global _NC_CACHE
    from concourse.bass_utils import run_bass_kernel_spmd

    if _NC_CACHE is None:
        _NC_CACHE = build_nc()
    in_maps = make_core_inputs(x, w_qkv, b_qkv, w_out, b_out)
    res = run_bass_kernel_spmd(_NC_CACHE, in_maps, core_ids=list(range(N_CORES)))
    return gather_output(res.results, b_out=b_out)
